# revision 4
# baseline (speedup 1.0000x reference)
"""CASSI forward A^T(A(x)) kernel for Trainium2, 8-core data parallel.

Reference computation (independent per batch b and row m):
    y1[l, n]  = x[b, l, m, n] * phi[l, m, n]
    y2[j]     = sum_l y1[l, j - 2l]              (j in [0, 310))
    out[l, n] = phi[l, m, n] * y2[2l + n]

On-chip layout: partitions = rows m (two 128-row tiles per batch), free
dim = (l, n).  The 28-band shift-scatter-add runs as a 5-level binary tree
of strided adds over scratch tiles laid out with small zero gaps between
paired bands, so each tree level is a wide strided tensor_tensor op whose
shifted operand reads zeros where a block has no data.  Gaps are memset
once at kernel start; level ops rewrite only data regions.

Uniform-slot layout: at every level, slot width = data width + next-level
shift, so in0's right-pad zeros and in1's left-pad zeros are the SAME gap
cells and every level op is a plain 2-free-dim strided tensor_tensor:
  y1  band l (256) at 258*l                        gaps [256,258) per slot
  u   i=0..13 (258) at 262*i                       gaps [258,262)
  q   i=0..6  (262) at 278*i                       gaps [262,278), [1930,1938)
  o   i=0..2  (270) at 286*i                       gaps [270,286), [842,850)
  s   s0 (286) at 0, m1 (278) at 342               zeros [286,342)
  y2  (310) dense

Engine split + software pipeline: the DVE runs the wide ops (mask-mul,
L1, L2, out-mul) in fp16 2x mode; the narrow levels L3-L5 run on the
otherwise-idle GpSimd engine (as 1-slot ops -- multi-slot strided APs
de-pipeline the Q7 read commands).  The out-mul for iteration i-1 is
emitted after iteration i's L2, so GpSimd's L3-L5(i) overlap DVE's
outmul(i-1)+mul(i+1) instead of serializing.  q and y2 are double
buffered because Tile dependency tracking is tile-granular.

Precision/layout strategy: all HBM traffic is fp16 (inputs are cast and
row-major-transposed to [.., M, L, N] on the host inside kernel(), the
output is cast/transposed back).  This halves HBM bytes (the kernel is
memory-bound), makes every DMA a dense contiguous transfer, and fp16
tensor_tensor runs in the DVE 2x perf mode (all scratch offsets are
even-element, keeping operands 4-byte aligned).  fp16 accumulation of 28
bands keeps worst-case relative error ~1e-3, far inside the 2e-2 gate.

Schedule: iteration 0's phi/x and iteration 1's x are split across both
HWDGE queues so the DVE starts ~4us earlier; the last iteration's
out-mul/store run as two band-halves on separate queues so the final
store drain overlaps the last DVE op.

Sharding: batch dim (32) split 4-per-core across 8 cores; phi replicated.
"""

import numpy as np

B, L, M, N = 32, 28, 256, 256
STRIDE = 2
NCORES = 8
BPC = B // NCORES            # batches per core
NOUT = N + STRIDE * (L - 1)  # 310
P = 128                      # partitions per row tile
LN = L * N                   # 7168
HALF = LN // 2               # 3584 (14 bands)
Y1_W = 258 * 28              # 7224, band l at 258*l, gaps [256,258) per slot
U_W = 262 * 14               # 3668, u_i at 262*i, gaps [258,262)
Q_W = 1938                   # q_i at 278*i (uniform); gaps [262,278), [1930,1938)
O_W = 850                    # o_i at 286*i; zeros [270,286)x2, [842,850)
S_W = 620                    # s0@0 (286), zeros [286,342), m1@342 (278)
GPSIMD_TAIL = True           # run tree levels 3-5 on GpSimd instead of DVE

_cached = {}


def _build_nc():
    import concourse.bass as bass
    import concourse.mybir as mybir
    from concourse.ap import AP
    from concourse.tile import TileContext

    f16 = mybir.dt.float16
    nc = bass.Bass()
    x = nc.dram_tensor("x", [BPC, M, LN], f16, kind="ExternalInput")
    phi = nc.dram_tensor("phi", [M, LN], f16, kind="ExternalInput")
    out = nc.dram_tensor("out", [BPC, M, LN], f16, kind="ExternalOutput")

    def sub(t, off, dims):
        """AP over tile t at element offset off with free dims [[step,count],..]."""
        full = t[:]
        return AP(full.tensor, full.offset + off,
                  [[full.ap[0][0], P]] + [list(d) for d in dims])

    def cols(hbm_ap, off, width):
        """Column slice [off, off+width) of a [128, LN] HBM access pattern."""
        return AP(hbm_ap.tensor, hbm_ap.offset + off,
                  [list(hbm_ap.ap[0]), [1, width]])

    with TileContext(nc) as tc:
        with (
            tc.tile_pool(name="phipool", bufs=1) as phipool,
            tc.tile_pool(name="xpool", bufs=1) as xpool,
            tc.tile_pool(name="scratch", bufs=1) as sp,
        ):
            tail = nc.gpsimd if GPSIMD_TAIL else nc.vector
            # --- persistent tiles ------------------------------------------------
            phit = [phipool.tile([P, LN], f16, name=f"phi{pt}", tag=f"phi{pt}")
                    for pt in range(M // P)]
            xts = [xpool.tile([P, LN], f16, name=f"xt{i}", tag=f"xt{i}")
                   for i in range(3)]
            x0h = [xpool.tile([P, HALF], f16, name=f"x0{i}", tag=f"x0{i}")
                   for i in range(2)]  # iteration 0's x, as two half tiles
            ots_ = [xpool.tile([P, LN], f16, name=f"ou{i}", tag=f"ou{i}")
                    for i in range(2)]
            y1t = sp.tile([P, Y1_W], f16, name="y1", tag="y1")
            ut = sp.tile([P, U_W], f16, name="u", tag="u")
            qts = [sp.tile([P, Q_W], f16, name=f"q{i}", tag=f"q{i}")
                   for i in range(2)]
            ot = sp.tile([P, O_W], f16, name="o", tag="o")
            st = sp.tile([P, S_W], f16, name="s", tag="s")
            y2s = [sp.tile([P, NOUT], f16, name=f"y2_{i}", tag=f"y2_{i}")
                   for i in range(2)]

            # --- one-time zero-gap memsets (never written afterwards) ------------
            nc.vector.memset(sub(y1t, 256, [[258, 28], [1, 2]]), 0.0)
            nc.vector.memset(sub(ut, 258, [[262, 14], [1, 4]]), 0.0)
            for qt in qts:
                nc.vector.memset(sub(qt, 262, [[278, 6], [1, 16]]), 0.0)
                nc.vector.memset(sub(qt, 1930, [[1, 8]]), 0.0)
            nc.vector.memset(sub(ot, 270, [[286, 2], [1, 16]]), 0.0)
            nc.vector.memset(sub(ot, 842, [[1, 8]]), 0.0)
            nc.vector.memset(sub(st, 286, [[1, 56]]), 0.0)

            # --- startup loads: iteration 0 split across both queues -------------
            nc.sync.dma_start(out=sub(phit[0], 0, [[1, HALF]]),
                              in_=cols(phi[0:P], 0, HALF))
            nc.scalar.dma_start(out=sub(phit[0], HALF, [[1, HALF]]),
                                in_=cols(phi[0:P], HALF, HALF))
            nc.sync.dma_start(out=x0h[0][:], in_=cols(x[0][0:P], 0, HALF))
            nc.scalar.dma_start(out=x0h[1][:], in_=cols(x[0][0:P], HALF, HALF))

            prev = None  # (outt, y2t, pt, o_hbm) of the previous iteration
            it = 0
            for pt in range(M // P):
                for b in range(BPC):
                    xt = xts[it % 3]
                    outt = ots_[it % 2]
                    y2t = y2s[it % 2]
                    qt = qts[it % 2]
                    it += 1
                    if it == 2:
                        # iteration 1's x split across both queues (the DVE
                        # is still ramping; a single queue would be late)
                        nc.sync.dma_start(out=sub(xt, 0, [[1, HALF]]),
                                          in_=cols(x[b][pt * P:(pt + 1) * P], 0, HALF))
                        nc.scalar.dma_start(out=sub(xt, HALF, [[1, HALF]]),
                                            in_=cols(x[b][pt * P:(pt + 1) * P], HALF, HALF))
                    elif it > 2:
                        nc.scalar.dma_start(
                            out=xt[:], in_=x[b][pt * P: (pt + 1) * P],
                        )
                    # y1 = x * phi, dense -> uniform gapped scratch
                    if it == 1:
                        for h in range(2):
                            nc.vector.tensor_mul(
                                out=sub(y1t, 258 * 14 * h, [[258, 14], [1, 256]]),
                                in0=sub(x0h[h], 0, [[256, 14], [1, 256]]),
                                in1=sub(phit[pt], HALF * h, [[256, 14], [1, 256]]),
                            )
                    else:
                        nc.vector.tensor_mul(
                            out=sub(y1t, 0, [[258, 28], [1, 256]]),
                            in0=sub(xt, 0, [[256, 28], [1, 256]]),
                            in1=sub(phit[pt], 0, [[256, 28], [1, 256]]),
                        )
                    # L1: 14 pair-sums -> u
                    nc.vector.tensor_add(
                        out=sub(ut, 0, [[262, 14], [1, 258]]),
                        in0=sub(y1t, 0, [[516, 14], [1, 258]]),
                        in1=sub(y1t, 256, [[516, 14], [1, 258]]),
                    )
                    # L2: 7 quad-sums -> q (single uniform op, stride 278)
                    nc.vector.tensor_add(
                        out=sub(qt, 0, [[278, 7], [1, 262]]),
                        in0=sub(ut, 0, [[524, 7], [1, 262]]),
                        in1=sub(ut, 258, [[524, 7], [1, 262]]),
                    )
                    # L3-L5 on GpSimd (1-slot ops), overlapping the DVE's
                    # outmul(i-1) and mul(i+1)
                    for k in range(3):
                        tail.tensor_add(
                            out=sub(ot, 286 * k, [[1, 270]]),
                            in0=sub(qt, 556 * k, [[1, 270]]),
                            in1=sub(qt, 556 * k + 270, [[1, 270]]),
                        )
                    tail.tensor_add(
                        out=sub(st, 0, [[1, 286]]),
                        in0=sub(ot, 0, [[1, 286]]),
                        in1=sub(ot, 270, [[1, 286]]),
                    )
                    tail.tensor_add(
                        out=sub(st, 342, [[1, 278]]),
                        in0=sub(ot, 572, [[1, 278]]),
                        in1=sub(qt, 1652, [[1, 278]]),
                    )
                    tail.tensor_add(
                        out=sub(y2t, 0, [[1, 310]]),
                        in0=sub(st, 0, [[1, 310]]),
                        in1=sub(st, 310, [[1, 310]]),
                    )
                    # previous iteration's out-mul + store (pipelined so the
                    # GpSimd tail above never sits on the DVE critical path)
                    if prev is not None:
                        poutt, py2t, ppt, po_hbm = prev
                        nc.vector.tensor_mul(
                            out=sub(poutt, 0, [[256, 28], [1, 256]]),
                            in0=sub(py2t, 0, [[2, 28], [1, 256]]),
                            in1=sub(phit[ppt], 0, [[256, 28], [1, 256]]),
                        )
                        nc.sync.dma_start(out=po_hbm, in_=poutt[:])
                        if it == 2:
                            # phi tile 1 in the store queue's idle window
                            nc.sync.dma_start(out=phit[1][:], in_=phi[P: 2 * P])
                    prev = (outt, y2t, pt, out[b][pt * P: (pt + 1) * P])

            # drain: last iteration's out-mul/store as two band-halves on
            # separate queues so the store drain overlaps the last DVE op
            poutt, py2t, ppt, po_hbm = prev
            for h, eng in ((0, nc.sync), (1, nc.scalar)):
                nc.vector.tensor_mul(
                    out=sub(poutt, HALF * h, [[256, 14], [1, 256]]),
                    in0=sub(py2t, 28 * h, [[2, 14], [1, 256]]),
                    in1=sub(phit[ppt], HALF * h, [[256, 14], [1, 256]]),
                )
                eng.dma_start(
                    out=cols(po_hbm, HALF * h, HALF),
                    in_=sub(poutt, HALF * h, [[1, HALF]]),
                )
    _split_excess_waits(nc, mybir)
    return nc


def _split_excess_waits(nc, mybir):
    """Move all-but-one semaphore waits off capacity-limited instructions.

    The TRN2 ISA packs sync commands into each 64B instruction; multi-dim
    TT/DMA encodings have room for only one wait, and walrus codegen dies
    with "Too many sync wait commands" instead of splitting.  A standalone
    EventSemaphore on the same engine right before the op is semantically
    identical (the sequencer executes both in order)."""
    ctr = 0
    for bb in nc.m.functions[0].blocks:
        new = []
        for ins in bb.instructions:
            si = ins.sync_info
            waits = list(si.on_wait) if si is not None and si.on_wait else []
            if len(waits) > 1:
                for w in waits[:-1]:
                    ctr += 1
                    new.append(mybir.InstEventSemaphore(
                        name=f"wsplit-{ctr}",
                        engine=ins.engine,
                        sync_info=mybir.SyncInfo(on_wait=[w], on_update=[]),
                    ))
                ins.sync_info = mybir.SyncInfo(
                    on_wait=[waits[-1]],
                    on_update=list(si.on_update or []),
                )
            new.append(ins)
        bb.instructions = new


def _get_nc():
    if "nc" not in _cached:
        _cached["nc"] = _build_nc()
    return _cached["nc"]


def _prep_in_maps(x: np.ndarray, phi: np.ndarray) -> list[dict]:
    """Shard batch across cores; cast to fp16 and transpose to [.., M, L, N]
    row-major so every device DMA is dense and contiguous."""
    phi_t = phi.transpose(1, 0, 2).reshape(M, LN).astype(np.float16, order="C")
    in_maps = []
    for c in range(NCORES):
        xs = (x[c * BPC: (c + 1) * BPC]
              .transpose(0, 2, 1, 3)
              .reshape(BPC, M, LN)
              .astype(np.float16, order="C"))
        in_maps.append({"x": xs, "phi": phi_t})
    return in_maps


def _postprocess(outs: list[np.ndarray]) -> np.ndarray:
    """Invert the device layout: fp16 [BPC, M, L*N] shards -> f32 [B,L,M,N]."""
    full = np.empty((B, L, M, N), dtype=np.float32)
    for c, o in enumerate(outs):
        o = np.asarray(o).reshape(BPC, M, L, N).astype(np.float32)
        full[c * BPC: (c + 1) * BPC] = o.transpose(0, 2, 1, 3)
    return full


def kernel(x: np.ndarray, phi: np.ndarray) -> np.ndarray:
    from concourse.bass_utils import run_bass_kernel_spmd

    x = np.asarray(x, dtype=np.float32)
    phi = np.asarray(phi, dtype=np.float32)
    assert x.shape == (B, L, M, N) and phi.shape == (L, M, N)

    nc = _get_nc()
    in_maps = _prep_in_maps(x, phi)
    res = run_bass_kernel_spmd(nc, in_maps, core_ids=list(range(NCORES)))
    return _postprocess([res.results[c]["out"] for c in range(NCORES)])


# revision 5
# speedup vs baseline: 1.3751x; 1.3751x over previous
"""CASSI forward A^T(A(x)) kernel for Trainium2, 8-core data parallel.

Reference computation (independent per batch b and row m):
    y1[l, n]  = x[b, l, m, n] * phi[l, m, n]
    y2[j]     = sum_l y1[l, j - 2l]              (j in [0, 310))
    out[l, n] = phi[l, m, n] * y2[2l + n]

On-chip layout: partitions = rows m (two 128-row tiles per batch), free
dim = (l, n).  The 28-band shift-scatter-add runs as a 5-level binary tree
of strided DVE adds over scratch tiles laid out with small zero gaps
between paired bands, so each tree level is a single wide strided
tensor_tensor op whose shifted operand reads zeros where a block has no
data.  Gaps are memset once at kernel start; level ops rewrite only data
regions.  Everything stays on the DVE: GpSimd shares the DVE's SBUF port
(concurrent Q7 traffic inflates every DVE op ~20%), so offloading tree
levels there is a net loss.

Uniform-slot layout: at every level, slot width = data width + next-level
shift, so in0's right-pad zeros and in1's left-pad zeros are the SAME gap
cells and every level op is a plain 2-free-dim strided tensor_tensor:
  y1  band l (256) at 258*l                        gaps [256,258) per slot
  u   i=0..13 (258) at 262*i                       gaps [258,262)
  q   i=0..6  (262) at 278*i                       gaps [262,278), [1930,1938)
  o   i=0..2  (270) at 286*i                       gaps [270,286), [842,850)
  s   s0 (286) at 0, m1 (278) at 342               zeros [286,342)
  y2  (310) dense

Precision/layout strategy: all HBM traffic is fp16 (inputs are cast and
row-major-transposed to [.., M, L, N] on the host inside kernel(), the
output is cast/transposed back).  This halves HBM bytes and DVE cycles
(fp16 tensor_tensor runs in the 2x perf mode; all scratch offsets are
even-element, keeping operands 4-byte aligned) and makes every DMA a
dense fully-contiguous transfer.  fp16 accumulation of 28 bands keeps
worst-case relative error ~1e-3, far inside the 2e-2 gate.

Schedule: iteration 0's x lands in two half-tiles and phi0 is split
column-wise, all four transfers spread over both HWDGE queues, so the
first mask-mul starts ~3.5us earlier (Tile dependency tracking is
tile-granular, so a *shared* tile would wait for both halves anyway).
Iteration 1's x is split across both queues to cover the ramp.  The last
iteration's out-mul/store run as two band-halves on separate queues so
the final store drain overlaps the last DVE op.

Sharding: batch dim (32) split 4-per-core across 8 cores; phi replicated.
"""

import numpy as np

B, L, M, N = 32, 28, 256, 256
STRIDE = 2
NCORES = 8
BPC = B // NCORES            # batches per core
NOUT = N + STRIDE * (L - 1)  # 310
P = 128                      # partitions per row tile
LN = L * N                   # 7168
HALF = LN // 2               # 3584 (14 bands)
Y1_W = 258 * 28              # 7224, band l at 258*l, gaps [256,258) per slot
U_W = 262 * 14               # 3668, u_i at 262*i, gaps [258,262)
Q_W = 1938                   # q_i at 278*i (uniform); gaps [262,278), [1930,1938)
O_W = 850                    # o_i at 286*i; zeros [270,286)x2, [842,850)
S_W = 620                    # s0@0 (286), zeros [286,342), m1@342 (278)

_cached = {}


def _build_nc():
    import concourse.bass as bass
    import concourse.mybir as mybir
    from concourse.ap import AP
    from concourse.tile import TileContext

    f16 = mybir.dt.float16
    nc = bass.Bass()
    x = nc.dram_tensor("x", [BPC, M, LN], f16, kind="ExternalInput")
    phi = nc.dram_tensor("phi", [M, LN], f16, kind="ExternalInput")
    out = nc.dram_tensor("out", [BPC, M, LN], f16, kind="ExternalOutput")

    def sub(t, off, dims):
        """AP over tile t at element offset off with free dims [[step,count],..]."""
        full = t[:]
        return AP(full.tensor, full.offset + off,
                  [[full.ap[0][0], P]] + [list(d) for d in dims])

    def cols(hbm_ap, off, width):
        """Column slice [off, off+width) of a [128, LN] HBM access pattern."""
        return AP(hbm_ap.tensor, hbm_ap.offset + off,
                  [list(hbm_ap.ap[0]), [1, width]])

    with TileContext(nc) as tc:
        with (
            tc.tile_pool(name="phipool", bufs=1) as phipool,
            tc.tile_pool(name="xpool", bufs=1) as xpool,
            tc.tile_pool(name="scratch", bufs=1) as sp,
        ):
            # --- persistent tiles ------------------------------------------------
            phit = [phipool.tile([P, LN], f16, name=f"phi{pt}", tag=f"phi{pt}")
                    for pt in range(M // P)]
            xts = [xpool.tile([P, LN], f16, name=f"xt{i}", tag=f"xt{i}")
                   for i in range(3)]
            x0h = [xpool.tile([P, HALF], f16, name=f"x0{i}", tag=f"x0{i}")
                   for i in range(2)]  # iteration 0's x, as two half tiles
            ots_ = [xpool.tile([P, LN], f16, name=f"ou{i}", tag=f"ou{i}")
                    for i in range(2)]
            y1t = sp.tile([P, Y1_W], f16, name="y1", tag="y1")
            ut = sp.tile([P, U_W], f16, name="u", tag="u")
            qt = sp.tile([P, Q_W], f16, name="q", tag="q")
            ot = sp.tile([P, O_W], f16, name="o", tag="o")
            st = sp.tile([P, S_W], f16, name="s", tag="s")
            y2t = sp.tile([P, NOUT], f16, name="y2", tag="y2")

            # --- one-time zero-gap memsets (never written afterwards) ------------
            nc.vector.memset(sub(y1t, 256, [[258, 28], [1, 2]]), 0.0)
            nc.vector.memset(sub(ut, 258, [[262, 14], [1, 4]]), 0.0)
            nc.vector.memset(sub(qt, 262, [[278, 6], [1, 16]]), 0.0)
            nc.vector.memset(sub(qt, 1930, [[1, 8]]), 0.0)
            nc.vector.memset(sub(ot, 270, [[286, 2], [1, 16]]), 0.0)
            nc.vector.memset(sub(ot, 842, [[1, 8]]), 0.0)
            nc.vector.memset(sub(st, 286, [[1, 56]]), 0.0)

            # --- startup loads: iteration 0 split across both queues -------------
            nc.sync.dma_start(out=sub(phit[0], 0, [[1, HALF]]),
                              in_=cols(phi[0:P], 0, HALF))
            nc.scalar.dma_start(out=sub(phit[0], HALF, [[1, HALF]]),
                                in_=cols(phi[0:P], HALF, HALF))
            nc.sync.dma_start(out=x0h[0][:], in_=cols(x[0][0:P], 0, HALF))
            nc.scalar.dma_start(out=x0h[1][:], in_=cols(x[0][0:P], HALF, HALF))

            it = 0
            for pt in range(M // P):
                for b in range(BPC):
                    xt = xts[it % 3]
                    outt = ots_[it % 2]
                    it += 1
                    if it == 2:
                        # iteration 1's x split across both queues (the DVE
                        # is still ramping; a single queue would be late)
                        nc.sync.dma_start(out=sub(xt, 0, [[1, HALF]]),
                                          in_=cols(x[b][pt * P:(pt + 1) * P], 0, HALF))
                        nc.scalar.dma_start(out=sub(xt, HALF, [[1, HALF]]),
                                            in_=cols(x[b][pt * P:(pt + 1) * P], HALF, HALF))
                    elif it > 2:
                        nc.scalar.dma_start(
                            out=xt[:], in_=x[b][pt * P: (pt + 1) * P],
                        )
                    # y1 = x * phi, dense -> uniform gapped scratch
                    if it == 1:
                        for h in range(2):
                            nc.vector.tensor_mul(
                                out=sub(y1t, 258 * 14 * h, [[258, 14], [1, 256]]),
                                in0=sub(x0h[h], 0, [[256, 14], [1, 256]]),
                                in1=sub(phit[pt], HALF * h, [[256, 14], [1, 256]]),
                            )
                    else:
                        nc.vector.tensor_mul(
                            out=sub(y1t, 0, [[258, 28], [1, 256]]),
                            in0=sub(xt, 0, [[256, 28], [1, 256]]),
                            in1=sub(phit[pt], 0, [[256, 28], [1, 256]]),
                        )
                    # L1: 14 pair-sums -> u
                    nc.vector.tensor_add(
                        out=sub(ut, 0, [[262, 14], [1, 258]]),
                        in0=sub(y1t, 0, [[516, 14], [1, 258]]),
                        in1=sub(y1t, 256, [[516, 14], [1, 258]]),
                    )
                    # L2: 7 quad-sums -> q (single uniform op, stride 278)
                    nc.vector.tensor_add(
                        out=sub(qt, 0, [[278, 7], [1, 262]]),
                        in0=sub(ut, 0, [[524, 7], [1, 262]]),
                        in1=sub(ut, 258, [[524, 7], [1, 262]]),
                    )
                    # L3: 3 oct-sums -> o
                    nc.vector.tensor_add(
                        out=sub(ot, 0, [[286, 3], [1, 270]]),
                        in0=sub(qt, 0, [[556, 3], [1, 270]]),
                        in1=sub(qt, 270, [[556, 3], [1, 270]]),
                    )
                    # L4: s0 = o0 + shift16(o1); m1 = o2 + shift16(q6)
                    nc.vector.tensor_add(
                        out=sub(st, 0, [[1, 286]]),
                        in0=sub(ot, 0, [[1, 286]]),
                        in1=sub(ot, 270, [[1, 286]]),
                    )
                    nc.vector.tensor_add(
                        out=sub(st, 342, [[1, 278]]),
                        in0=sub(ot, 572, [[1, 278]]),
                        in1=sub(qt, 1652, [[1, 278]]),
                    )
                    # L5: y2 = s0 + shift32(m1)
                    nc.vector.tensor_add(
                        out=sub(y2t, 0, [[1, 310]]),
                        in0=sub(st, 0, [[1, 310]]),
                        in1=sub(st, 310, [[1, 310]]),
                    )
                    # out = phi * gather(y2) into a dense tile so the store
                    # is a single fully-contiguous transfer
                    o_hbm = out[b][pt * P: (pt + 1) * P]
                    if it < 2 * BPC:
                        nc.vector.tensor_mul(
                            out=sub(outt, 0, [[256, 28], [1, 256]]),
                            in0=sub(y2t, 0, [[2, 28], [1, 256]]),
                            in1=sub(phit[pt], 0, [[256, 28], [1, 256]]),
                        )
                        # full store on the SP ring (ACT ring carries loads)
                        nc.sync.dma_start(out=o_hbm, in_=outt[:])
                        if it == 1:
                            # phi tile 1 in the store queue's idle window
                            nc.sync.dma_start(out=phit[1][:], in_=phi[P: 2 * P])
                    else:
                        # last iteration: two half-band out-muls, each store
                        # firing as its half completes (on separate rings)
                        for h, eng in ((0, nc.sync), (1, nc.scalar)):
                            nc.vector.tensor_mul(
                                out=sub(outt, HALF * h, [[256, 14], [1, 256]]),
                                in0=sub(y2t, 28 * h, [[2, 14], [1, 256]]),
                                in1=sub(phit[pt], HALF * h, [[256, 14], [1, 256]]),
                            )
                            eng.dma_start(
                                out=cols(o_hbm, HALF * h, HALF),
                                in_=sub(outt, HALF * h, [[1, HALF]]),
                            )
    _split_excess_waits(nc, mybir)
    return nc


def _split_excess_waits(nc, mybir):
    """Move all-but-one semaphore waits off capacity-limited instructions.

    The TRN2 ISA packs sync commands into each 64B instruction; multi-dim
    TT/DMA encodings have room for only one wait, and walrus codegen dies
    with "Too many sync wait commands" instead of splitting.  A standalone
    EventSemaphore on the same engine right before the op is semantically
    identical (the sequencer executes both in order)."""
    ctr = 0
    for bb in nc.m.functions[0].blocks:
        new = []
        for ins in bb.instructions:
            si = ins.sync_info
            waits = list(si.on_wait) if si is not None and si.on_wait else []
            if len(waits) > 1:
                for w in waits[:-1]:
                    ctr += 1
                    new.append(mybir.InstEventSemaphore(
                        name=f"wsplit-{ctr}",
                        engine=ins.engine,
                        sync_info=mybir.SyncInfo(on_wait=[w], on_update=[]),
                    ))
                ins.sync_info = mybir.SyncInfo(
                    on_wait=[waits[-1]],
                    on_update=list(si.on_update or []),
                )
            new.append(ins)
        bb.instructions = new


def _get_nc():
    if "nc" not in _cached:
        _cached["nc"] = _build_nc()
    return _cached["nc"]


def _prep_in_maps(x: np.ndarray, phi: np.ndarray) -> list[dict]:
    """Shard batch across cores; cast to fp16 and transpose to [.., M, L, N]
    row-major so every device DMA is dense and contiguous."""
    phi_t = phi.transpose(1, 0, 2).reshape(M, LN).astype(np.float16, order="C")
    in_maps = []
    for c in range(NCORES):
        xs = (x[c * BPC: (c + 1) * BPC]
              .transpose(0, 2, 1, 3)
              .reshape(BPC, M, LN)
              .astype(np.float16, order="C"))
        in_maps.append({"x": xs, "phi": phi_t})
    return in_maps


def _postprocess(outs: list[np.ndarray]) -> np.ndarray:
    """Invert the device layout: fp16 [BPC, M, L*N] shards -> f32 [B,L,M,N]."""
    full = np.empty((B, L, M, N), dtype=np.float32)
    for c, o in enumerate(outs):
        o = np.asarray(o).reshape(BPC, M, L, N).astype(np.float32)
        full[c * BPC: (c + 1) * BPC] = o.transpose(0, 2, 1, 3)
    return full


def kernel(x: np.ndarray, phi: np.ndarray) -> np.ndarray:
    from concourse.bass_utils import run_bass_kernel_spmd

    x = np.asarray(x, dtype=np.float32)
    phi = np.asarray(phi, dtype=np.float32)
    assert x.shape == (B, L, M, N) and phi.shape == (L, M, N)

    nc = _get_nc()
    in_maps = _prep_in_maps(x, phi)
    res = run_bass_kernel_spmd(nc, in_maps, core_ids=list(range(NCORES)))
    return _postprocess([res.results[c]["out"] for c in range(NCORES)])
